# revision 19
# baseline (speedup 1.0000x reference)
"""CenterLoss Trainium2 kernel.

Full inputs:
  ep_mask_embed    (8, 4096, 256) f32
  ep_mask          (8, 1, 1024, 1024) f32
  query_mask_embed (8, 4096, 256) f32
  query_mask       (8, 1, 1024, 1024) f32
Output: (3,) f32 = [mean(center_loss), mean(pos_loss), mean(neg_loss)]

Sharding: data-parallel, one batch sample per NeuronCore (8 cores).

Math (per sample, c=256, N=4096, m = mask downsampled to (N,)):
  PSUM-accumulated bf16 matmul chains (lhsT = [m, 1-m] mask columns,
  token-on-partition, contraction over 128 tokens per group):
    psum_ew [2,256] += [ep_m,1-ep_m]^T @ ep_embed
    psum_q  [2,512] += [q_m, 1-q_m]^T @ [q_embed | q_embed^2]   (fused N=512)
  The tiny accumulators (counts [2,2], [qw|qsw] [2,512], ew [2,256])
  are exported and the final ~1K-flop epilogue runs on host in f64:
    Ctr = ew/(n_ep+0.1)
    num = sum(qsw) - 2*dot(Ctr,qw) + n_q*dot(Ctr,Ctr)
    loss = num / (max(n_q,1)*c) * min(n_q,1)   per [pos;neg] row.

Performance structure (per-core stream is HBM-bound at ~410 GB/s, 8.4MB
=> ~20.5us floor; measured preamble ~6.6us and post-output ~4us are
framework-fixed):
  - t=32 token staging: partition p holds tokens 32p..32p+31, so every
    512KB chunk DMA is 128 descriptors x 4KB contiguous; mask weight
    columns are a plain host-side reshape.
  - ALL stream DMAs are emitted first on the sync HWDGE queue
    (lm, q0..7, ep0..7) so descriptor generation runs ahead and the
    SDMA backlog holds the stream at line rate.
  - q streams FIRST: its heavy chase (DVE cast + ACT/DVE squares +
    4 fused N=512 matmuls per 512KB chunk) hides under the stream
    middle, and the ep tail work after the last chunk lands is tiny
    (two half-casts + 4 N=256 matmuls + one bank-merge + export DMA).
  - each matmul chain accumulates into TWO alternating PSUM banks:
    back-to-back matmuls into one bank serialize at full fill+drain
    latency ((398+N)/2.4 ns); alternating banks restores the ~N/2.4
    streaming rate.  Banks are merged with one DVE add off the
    critical path (q) / on a short tail (ep).
  - counts export early, the merged q accumulator exports mid-stream;
    only the ep bank-merge + one 1KB DMA remain after the last matmul.
The host shards, downsamples (stride-16 indexing), and reshapes the
mask values per sample, then runs the (np, f64) final epilogue on the
8 exported accumulator sets.
"""

import numpy as np
from contextlib import ExitStack

import concourse.bass as bass
import concourse.bacc as bacc
import concourse.tile as tile
from concourse import mybir
from concourse.bass_utils import run_bass_kernel_spmd

F32 = mybir.dt.float32
BF16 = mybir.dt.bfloat16

P = 128          # partitions
N_TOK = 4096     # tokens per sample (64*64 patches)
C = 256          # channels
T = 32           # tokens per partition (whole tensor)
NG = T           # 32 token groups of 128 tokens
GPC = 4          # groups per 512KB chunk
Q_CH = [16, 16]                # q chunk sizes in groups (2 x 2MB):
                               # deep SDMA backlog from the first issue
EP_CH = [16, 8, 4, 2, 1, 1]    # ep chunks taper to 128KB so the
                               # post-stream tail is one group
N_CH = 8         # legacy (pool sizing)
B = 8            # batch == n cores
PATCH = 16

_CACHE = {}


def _build():
    """Build the per-core Bass program (identical on all cores)."""
    nc = bacc.Bacc("TRN2", target_bir_lowering=False, debug=False)

    ep_embed = nc.dram_tensor("ep_embed", [N_TOK, C], F32, kind="ExternalInput").ap()
    q_embed = nc.dram_tensor("q_embed", [N_TOK, C], F32, kind="ExternalInput").ap()
    # downsampled mask values in weight-column layout:
    # lm[p, g] = mask_ds[32p + g]; cols 0:32 = ep mask, cols 32:64 = q mask
    lm = nc.dram_tensor("lm", [P, 2 * NG], F32, kind="ExternalInput").ap()
    # raw per-core accumulators; the final ~1K flops run on host
    out_cnt = nc.dram_tensor("out_cnt", [2, 2], F32, kind="ExternalOutput").ap()
    out_q = nc.dram_tensor("out_q", [2, 512], F32, kind="ExternalOutput").ap()
    out_ew = nc.dram_tensor("out_ew", [2, 256], F32, kind="ExternalOutput").ap()

    AF = mybir.ActivationFunctionType
    OP = mybir.AluOpType

    ep_src = ep_embed.rearrange("(p t) c -> p (t c)", t=T)   # [128, 32*256]
    q_src = q_embed.rearrange("(p t) c -> p (t c)", t=T)
    W = GPC * C                                              # 1024 f32 / chunk

    with tile.TileContext(nc) as tc, ExitStack() as ctx:
        const_pool = ctx.enter_context(tc.tile_pool(name="const", bufs=1))
        q_f = ctx.enter_context(tc.tile_pool(name="q_f", bufs=1))
        q_b = ctx.enter_context(tc.tile_pool(name="q_b", bufs=1))
        ep_f = ctx.enter_context(tc.tile_pool(name="ep_f", bufs=1))
        ep_b = ctx.enter_context(tc.tile_pool(name="ep_b", bufs=1))
        psum_pool = ctx.enter_context(
            tc.tile_pool(name="psum", bufs=1, space=bass.MemorySpace.PSUM)
        )
        fin_pool = ctx.enter_context(tc.tile_pool(name="fin", bufs=1))

        # ---- ALL stream DMAs first: they sit on the sync HWDGE queue in
        # this order and drain back-to-back at line rate ----
        lm_t = const_pool.tile([P, 2 * NG], F32, name="lm_t", tag="lm_t")
        nc.scalar.dma_start(out=lm_t[:], in_=lm[:])

        tq = []
        off = 0
        for j, gs in enumerate(Q_CH):
            t_ = q_f.tile([P, gs * C], F32, name=f"tq{j}", tag=f"tq{j}")
            nc.sync.dma_start(out=t_[:], in_=q_src[:, off * C:(off + gs) * C])
            tq.append((t_, off, gs))
            off += gs
        te = []
        off = 0
        for j, gs in enumerate(EP_CH):
            t_ = ep_f.tile([P, gs * C], F32, name=f"te{j}", tag=f"te{j}")
            nc.sync.dma_start(out=t_[:], in_=ep_src[:, off * C:(off + gs) * C])
            te.append((t_, off, gs))
            off += gs

        # ---- mask prep: per tensor L64 [128, 64] = [m (32) | 1-m (32)];
        # lhsT for group g = cols {g, g+32} (free stride 32) ----
        L = {}
        for li, nm in enumerate(("ep", "q")):
            L64 = const_pool.tile([P, 2 * NG], F32, name=f"L64_{nm}", tag=f"L64_{nm}")
            nc.vector.tensor_copy(L64[:, 0:NG], lm_t[:, li * NG:(li + 1) * NG])
            nc.vector.tensor_scalar(
                out=L64[:, NG:2 * NG], in0=L64[:, 0:NG], scalar1=-1.0,
                scalar2=1.0, op0=OP.mult, op1=OP.add,
            )
            Lb = const_pool.tile([P, 2 * NG], BF16, name=f"Lb_{nm}", tag=f"Lb_{nm}")
            nc.vector.tensor_copy(Lb[:], L64[:])
            L[nm] = Lb
            # per-partition mask sums -> [pos, neg] counts via a tiny matmul
            rs = const_pool.tile([P, 2], F32, name=f"rs_{nm}", tag=f"rs_{nm}")
            nc.vector.tensor_reduce(
                rs[:, 0:1], L64[:, 0:NG], axis=mybir.AxisListType.X, op=OP.add)
            nc.vector.tensor_reduce(
                rs[:, 1:2], L64[:, NG:2 * NG], axis=mybir.AxisListType.X,
                op=OP.add)
            L[nm + "_rs"] = rs

        ones1 = const_pool.tile([P, 1], F32, name="ones1", tag="ones1")
        nc.vector.memset(ones1[:], 1.0)

        def lhsT(nm, g):
            # 2-column AP [m, 1-m] with free stride NG
            return L[nm].rearrange("p (h c) -> p c h", h=2)[:, g, :]

        # PSUM accumulators, two banks per chain (pos=partition 0, neg=1).
        # Full-bank [2, 512] allocations keep the two banks of a chain in
        # physically distinct PSUM banks so matmuls pipeline.
        psum_q = [
            psum_pool.tile([2, 2 * C], F32, name=f"psum_q{b_}", tag=f"pq{b_}")
            for b_ in range(2)
        ]
        # ep chain is single-bank: latency-rate N=256 matmuls (272ns)
        # still outpace the stream, and one bank means the tail needs
        # only one PSUM->SBUF copy, no merge add
        psum_ewb = psum_pool.tile([2, 2 * C], F32, name="psum_ewb", tag="pewb")
        psum_en = psum_pool.tile([2, 1], F32, name="psum_en", tag="pen")
        psum_qn = psum_pool.tile([2, 1], F32, name="psum_qn", tag="pqn")
        nc.tensor.matmul(psum_en[:], L["ep_rs"][:], ones1[:])
        nc.tensor.matmul(psum_qn[:], L["q_rs"][:], ones1[:])

        # counts exported early (sync queue, behind the stream DGEs)
        cnt = fin_pool.tile([2, 2], F32, name="cnt", tag="cnt")
        nc.vector.tensor_copy(cnt[:, 0:1], psum_en[:, 0:1])
        nc.vector.tensor_copy(cnt[:, 1:2], psum_qn[:, 0:1])
        nc.sync.dma_start(out=out_cnt[:], in_=cnt[:])

        # ---- q half (first): DVE casts q, squares split ACT (5/8 from
        # f32) / DVE (3/8 from bf16); fused [q | q^2] N=512 matmuls ----
        for j, (t_, off, gs) in enumerate(tq):
            w = gs * C
            sqa = (w * 5) // 8 // 256 * 256     # ACT share of the squares
            qb = q_b.tile([P, 2 * w], BF16, name=f"qb{j}", tag=f"qb{j}")
            nc.vector.tensor_copy(qb[:, 0:w], t_[:])
            nc.scalar.activation(
                out=qb[:, w:w + sqa], in_=t_[:, 0:sqa], func=AF.Square)
            nc.vector.tensor_mul(
                qb[:, w + sqa:2 * w], qb[:, sqa:w], qb[:, sqa:w])
            # [128, 2, 256] rhs AP: blocks {q, q^2} for group g, streamed
            # as 512 free-dim elements matching psum_q columns
            qb2 = qb.rearrange("p (h gc) -> p h gc", h=2)
            for g in range(gs):
                idx = off + g
                nc.tensor.matmul(
                    psum_q[idx % 2][:], lhsT("q", idx),
                    qb2[:, :, g * C:(g + 1) * C],
                    start=(idx < 2), stop=(idx >= NG - 2),
                )

        # ---- mid-stream: merge q banks and export [qw | qsw] ----
        qw_s = fin_pool.tile([2, 2 * C], F32, name="qw_s", tag="qw_s")
        nc.vector.tensor_copy(qw_s[:], psum_q[0][:])
        nc.vector.tensor_add(qw_s[:], qw_s[:], psum_q[1][:])
        nc.sync.dma_start(out=out_q[:], in_=qw_s[:])

        # ---- ep half (second): cast split DVE (5/8) / ACT (3/8),
        # N=256 matmuls on alternating banks ----
        for j, (t_, off, gs) in enumerate(te):
            w = gs * C
            # DVE takes ~5/8 (it casts ~1.7x faster than ACT); whole
            # chunk on DVE when the ACT share rounds away
            h = min(w, max(C, round(w * 5 / 8 / C) * C))
            rb = ep_b.tile([P, w], BF16, name=f"re{j}", tag=f"re{j}")
            nc.vector.tensor_copy(rb[:, 0:h], t_[:, 0:h])
            if h < w:
                nc.scalar.copy(rb[:, h:w], t_[:, h:w])
            for g in range(gs):
                idx = off + g
                nc.tensor.matmul(
                    psum_ewb[:, 0:C], lhsT("ep", idx),
                    rb[:, g * C:(g + 1) * C],
                    start=(idx == 0), stop=(idx == NG - 1),
                )

        # ---- tail: one PSUM->SBUF copy + export (epilogue on host) ----
        eww = fin_pool.tile([2, C], F32, name="eww", tag="eww")
        nc.vector.tensor_copy(eww[:], psum_ewb[:, 0:C])
        nc.sync.dma_start(out=out_ew[:], in_=eww[:])

    nc.compile()
    return nc


def get_nc():
    if "nc" not in _CACHE:
        _CACHE["nc"] = _build()
    return _CACHE["nc"]


def _perm_mask(mask_b):
    """Downsampled mask in the kernel's weight-column layout:
    Lm[p, g] = ds_flat[32p + g] (plain reshape)."""
    ds = mask_b[0, ::PATCH, ::PATCH].reshape(-1)           # (4096,)
    return np.ascontiguousarray(ds.reshape(P, T))


def make_in_maps(ep_mask_embed, ep_mask, query_mask_embed, query_mask):
    in_maps = []
    for b in range(B):
        in_maps.append({
            "ep_embed": np.ascontiguousarray(ep_mask_embed[b]),
            "q_embed": np.ascontiguousarray(query_mask_embed[b]),
            "lm": np.concatenate(
                [_perm_mask(ep_mask[b]), _perm_mask(query_mask[b])], axis=1),
        })
    return in_maps


def finalize(results):
    """results: list of 8 dicts with out_cnt [2,2], out_q [2,512],
    out_ew [2,256] -> full (3,) output (final epilogue in float64)."""
    pos = np.zeros(B)
    neg = np.zeros(B)
    for b, r in enumerate(results):
        cnt = np.asarray(r["out_cnt"], dtype=np.float64)   # [:,0]=ep, [:,1]=q
        qws = np.asarray(r["out_q"], dtype=np.float64)     # [qw | qsw]
        ew = np.asarray(r["out_ew"], dtype=np.float64)
        qw, qsw = qws[:, 0:C], qws[:, C:2 * C]
        ctr = ew / (cnt[:, 0:1] + 0.1)
        num = (qsw.sum(1) - 2.0 * (ctr * qw).sum(1)
               + cnt[:, 1] * (ctr * ctr).sum(1))
        loss = num / (np.maximum(cnt[:, 1], 1.0) * C) * np.minimum(cnt[:, 1], 1.0)
        pos[b], neg[b] = loss[0], loss[1]
    return np.array(
        [(pos + neg).mean(), pos.mean(), neg.mean()], dtype=np.float32
    )


def kernel(ep_mask_embed, ep_mask, query_mask_embed, query_mask):
    ep_mask_embed = np.asarray(ep_mask_embed, dtype=np.float32)
    ep_mask = np.asarray(ep_mask, dtype=np.float32)
    query_mask_embed = np.asarray(query_mask_embed, dtype=np.float32)
    query_mask = np.asarray(query_mask, dtype=np.float32)

    nc = get_nc()
    in_maps = make_in_maps(ep_mask_embed, ep_mask, query_mask_embed, query_mask)
    res = run_bass_kernel_spmd(nc, in_maps, list(range(B)))
    return finalize(res.results)


# revision 20
# speedup vs baseline: 1.0316x; 1.0316x over previous
"""CenterLoss Trainium2 kernel.

Full inputs:
  ep_mask_embed    (8, 4096, 256) f32
  ep_mask          (8, 1, 1024, 1024) f32
  query_mask_embed (8, 4096, 256) f32
  query_mask       (8, 1, 1024, 1024) f32
Output: (3,) f32 = [mean(center_loss), mean(pos_loss), mean(neg_loss)]

Sharding: data-parallel, one batch sample per NeuronCore (8 cores).

Math (per sample, c=256, N=4096, m = mask downsampled to (N,)):
  PSUM-accumulated bf16 matmul chains (lhsT = [m, 1-m] mask columns,
  token-on-partition, contraction over 128 tokens per group):
    psum_ew [2,256] += [ep_m,1-ep_m]^T @ ep_embed
    psum_q  [2,512] += [q_m, 1-q_m]^T @ [q_embed | q_embed^2]   (fused N=512)
  The tiny accumulators (counts [2,2], [qw|qsw] [2,512], ew [2,256])
  are exported and the final ~1K-flop epilogue runs on host in f64:
    Ctr = ew/(n_ep+0.1)
    num = sum(qsw) - 2*dot(Ctr,qw) + n_q*dot(Ctr,Ctr)
    loss = num / (max(n_q,1)*c) * min(n_q,1)   per [pos;neg] row.

Performance structure (per-core stream is HBM-bound at ~410 GB/s, 8.4MB
=> ~20.5us floor; measured preamble ~6.6us and post-output ~4us are
framework-fixed):
  - t=32 token staging: partition p holds tokens 32p..32p+31, so every
    512KB chunk DMA is 128 descriptors x 4KB contiguous; mask weight
    columns are a plain host-side reshape.
  - ALL stream DMAs are emitted first on the sync HWDGE queue
    (lm, q0..7, ep0..7) so descriptor generation runs ahead and the
    SDMA backlog holds the stream at line rate.
  - q streams FIRST: its heavy chase (DVE cast + ACT/DVE squares +
    4 fused N=512 matmuls per 512KB chunk) hides under the stream
    middle, and the ep tail work after the last chunk lands is tiny
    (two half-casts + 4 N=256 matmuls + one bank-merge + export DMA).
  - each matmul chain accumulates into TWO alternating PSUM banks:
    back-to-back matmuls into one bank serialize at full fill+drain
    latency ((398+N)/2.4 ns); alternating banks restores the ~N/2.4
    streaming rate.  Banks are merged with one DVE add off the
    critical path (q) / on a short tail (ep).
  - counts export early, the merged q accumulator exports mid-stream;
    only the ep bank-merge + one 1KB DMA remain after the last matmul.
The host shards, downsamples (stride-16 indexing), and reshapes the
mask values per sample, then runs the (np, f64) final epilogue on the
8 exported accumulator sets.
"""

import numpy as np
from contextlib import ExitStack

import concourse.bass as bass
import concourse.bacc as bacc
import concourse.tile as tile
from concourse import mybir
from concourse.bass_utils import run_bass_kernel_spmd

F32 = mybir.dt.float32
BF16 = mybir.dt.bfloat16

P = 128          # partitions
N_TOK = 4096     # tokens per sample (64*64 patches)
C = 256          # channels
T = 32           # tokens per partition (whole tensor)
NG = T           # 32 token groups of 128 tokens
GPC = 4          # groups per 512KB chunk
Q_CH = [8, 8, 8, 8]            # q chunk sizes in groups (4 x 1MB);
                               # bigger chunks make the cast chase lumpy
EP_CH = [8, 8, 4, 4, 4, 3, 1]  # ep chunks taper to 128KB so the
                               # post-stream tail is one group
N_CH = 8         # legacy (pool sizing)
B = 8            # batch == n cores
PATCH = 16

_CACHE = {}


def _build():
    """Build the per-core Bass program (identical on all cores)."""
    nc = bacc.Bacc("TRN2", target_bir_lowering=False, debug=False)

    ep_embed = nc.dram_tensor("ep_embed", [N_TOK, C], F32, kind="ExternalInput").ap()
    q_embed = nc.dram_tensor("q_embed", [N_TOK, C], F32, kind="ExternalInput").ap()
    # downsampled mask values in weight-column layout:
    # lm[p, g] = mask_ds[32p + g]; cols 0:32 = ep mask, cols 32:64 = q mask
    lm = nc.dram_tensor("lm", [P, 2 * NG], F32, kind="ExternalInput").ap()
    # raw per-core accumulators; the final ~1K flops run on host
    out_cnt = nc.dram_tensor("out_cnt", [2, 2], F32, kind="ExternalOutput").ap()
    out_q = nc.dram_tensor("out_q", [2, 512], F32, kind="ExternalOutput").ap()
    out_ew = nc.dram_tensor("out_ew", [2, 256], F32, kind="ExternalOutput").ap()

    AF = mybir.ActivationFunctionType
    OP = mybir.AluOpType

    ep_src = ep_embed.rearrange("(p t) c -> p (t c)", t=T)   # [128, 32*256]
    q_src = q_embed.rearrange("(p t) c -> p (t c)", t=T)
    W = GPC * C                                              # 1024 f32 / chunk

    with tile.TileContext(nc) as tc, ExitStack() as ctx:
        const_pool = ctx.enter_context(tc.tile_pool(name="const", bufs=1))
        q_f = ctx.enter_context(tc.tile_pool(name="q_f", bufs=1))
        q_b = ctx.enter_context(tc.tile_pool(name="q_b", bufs=1))
        ep_f = ctx.enter_context(tc.tile_pool(name="ep_f", bufs=1))
        ep_b = ctx.enter_context(tc.tile_pool(name="ep_b", bufs=1))
        psum_pool = ctx.enter_context(
            tc.tile_pool(name="psum", bufs=1, space=bass.MemorySpace.PSUM)
        )
        fin_pool = ctx.enter_context(tc.tile_pool(name="fin", bufs=1))

        # ---- ALL stream DMAs first: they sit on the sync HWDGE queue in
        # this order and drain back-to-back at line rate ----
        lm_t = const_pool.tile([P, 2 * NG], F32, name="lm_t", tag="lm_t")
        nc.scalar.dma_start(out=lm_t[:], in_=lm[:])

        tq = []
        off = 0
        for j, gs in enumerate(Q_CH):
            t_ = q_f.tile([P, gs * C], F32, name=f"tq{j}", tag=f"tq{j}")
            nc.sync.dma_start(out=t_[:], in_=q_src[:, off * C:(off + gs) * C])
            tq.append((t_, off, gs))
            off += gs
        te = []
        off = 0
        for j, gs in enumerate(EP_CH):
            t_ = ep_f.tile([P, gs * C], F32, name=f"te{j}", tag=f"te{j}")
            nc.sync.dma_start(out=t_[:], in_=ep_src[:, off * C:(off + gs) * C])
            te.append((t_, off, gs))
            off += gs

        # ---- mask prep: per tensor L64 [128, 64] = [m (32) | 1-m (32)];
        # lhsT for group g = cols {g, g+32} (free stride 32) ----
        L = {}
        for li, nm in enumerate(("ep", "q")):
            L64 = const_pool.tile([P, 2 * NG], F32, name=f"L64_{nm}", tag=f"L64_{nm}")
            nc.vector.tensor_copy(L64[:, 0:NG], lm_t[:, li * NG:(li + 1) * NG])
            nc.vector.tensor_scalar(
                out=L64[:, NG:2 * NG], in0=L64[:, 0:NG], scalar1=-1.0,
                scalar2=1.0, op0=OP.mult, op1=OP.add,
            )
            Lb = const_pool.tile([P, 2 * NG], BF16, name=f"Lb_{nm}", tag=f"Lb_{nm}")
            nc.vector.tensor_copy(Lb[:], L64[:])
            L[nm] = Lb
            # per-partition mask sums -> [pos, neg] counts via a tiny matmul
            rs = const_pool.tile([P, 2], F32, name=f"rs_{nm}", tag=f"rs_{nm}")
            nc.vector.tensor_reduce(
                rs[:, 0:1], L64[:, 0:NG], axis=mybir.AxisListType.X, op=OP.add)
            nc.vector.tensor_reduce(
                rs[:, 1:2], L64[:, NG:2 * NG], axis=mybir.AxisListType.X,
                op=OP.add)
            L[nm + "_rs"] = rs

        ones1 = const_pool.tile([P, 1], F32, name="ones1", tag="ones1")
        nc.vector.memset(ones1[:], 1.0)

        def lhsT(nm, g):
            # 2-column AP [m, 1-m] with free stride NG
            return L[nm].rearrange("p (h c) -> p c h", h=2)[:, g, :]

        # PSUM accumulators, two banks per chain (pos=partition 0, neg=1).
        # Full-bank [2, 512] allocations keep the two banks of a chain in
        # physically distinct PSUM banks so matmuls pipeline.
        psum_q = [
            psum_pool.tile([2, 2 * C], F32, name=f"psum_q{b_}", tag=f"pq{b_}")
            for b_ in range(2)
        ]
        # ep chain is single-bank: latency-rate N=256 matmuls (272ns)
        # still outpace the stream, and one bank means the tail needs
        # only one PSUM->SBUF copy, no merge add
        psum_ewb = psum_pool.tile([2, 2 * C], F32, name="psum_ewb", tag="pewb")
        psum_en = psum_pool.tile([2, 1], F32, name="psum_en", tag="pen")
        psum_qn = psum_pool.tile([2, 1], F32, name="psum_qn", tag="pqn")
        nc.tensor.matmul(psum_en[:], L["ep_rs"][:], ones1[:])
        nc.tensor.matmul(psum_qn[:], L["q_rs"][:], ones1[:])

        # counts exported early (sync queue, behind the stream DGEs)
        cnt = fin_pool.tile([2, 2], F32, name="cnt", tag="cnt")
        nc.vector.tensor_copy(cnt[:, 0:1], psum_en[:, 0:1])
        nc.vector.tensor_copy(cnt[:, 1:2], psum_qn[:, 0:1])
        nc.sync.dma_start(out=out_cnt[:], in_=cnt[:])

        # ---- q half (first): DVE casts q, squares split ACT (5/8 from
        # f32) / DVE (3/8 from bf16); fused [q | q^2] N=512 matmuls ----
        for j, (t_, off, gs) in enumerate(tq):
            w = gs * C
            sqa = (w * 5) // 8 // 256 * 256     # ACT share of the squares
            qb = q_b.tile([P, 2 * w], BF16, name=f"qb{j}", tag=f"qb{j}")
            nc.vector.tensor_copy(qb[:, 0:w], t_[:])
            nc.scalar.activation(
                out=qb[:, w:w + sqa], in_=t_[:, 0:sqa], func=AF.Square)
            nc.vector.tensor_mul(
                qb[:, w + sqa:2 * w], qb[:, sqa:w], qb[:, sqa:w])
            # [128, 2, 256] rhs AP: blocks {q, q^2} for group g, streamed
            # as 512 free-dim elements matching psum_q columns
            qb2 = qb.rearrange("p (h gc) -> p h gc", h=2)
            for g in range(gs):
                idx = off + g
                nc.tensor.matmul(
                    psum_q[idx % 2][:], lhsT("q", idx),
                    qb2[:, :, g * C:(g + 1) * C],
                    start=(idx < 2), stop=(idx >= NG - 2),
                )

        # ---- mid-stream: merge q banks and export [qw | qsw] ----
        qw_s = fin_pool.tile([2, 2 * C], F32, name="qw_s", tag="qw_s")
        nc.vector.tensor_copy(qw_s[:], psum_q[0][:])
        nc.vector.tensor_add(qw_s[:], qw_s[:], psum_q[1][:])
        nc.sync.dma_start(out=out_q[:], in_=qw_s[:])

        # ---- ep half (second): cast split DVE (5/8) / ACT (3/8),
        # N=256 matmuls on alternating banks ----
        for j, (t_, off, gs) in enumerate(te):
            w = gs * C
            # DVE takes ~5/8 (it casts ~1.7x faster than ACT); whole
            # chunk on DVE when the ACT share rounds away
            h = min(w, max(C, round(w * 5 / 8 / C) * C))
            rb = ep_b.tile([P, w], BF16, name=f"re{j}", tag=f"re{j}")
            nc.vector.tensor_copy(rb[:, 0:h], t_[:, 0:h])
            if h < w:
                nc.scalar.copy(rb[:, h:w], t_[:, h:w])
            for g in range(gs):
                idx = off + g
                nc.tensor.matmul(
                    psum_ewb[:, 0:C], lhsT("ep", idx),
                    rb[:, g * C:(g + 1) * C],
                    start=(idx == 0), stop=(idx == NG - 1),
                )

        # ---- tail: one PSUM->SBUF copy + export (epilogue on host) ----
        eww = fin_pool.tile([2, C], F32, name="eww", tag="eww")
        nc.vector.tensor_copy(eww[:], psum_ewb[:, 0:C])
        nc.sync.dma_start(out=out_ew[:], in_=eww[:])

    nc.compile()
    return nc


def get_nc():
    if "nc" not in _CACHE:
        _CACHE["nc"] = _build()
    return _CACHE["nc"]


def _perm_mask(mask_b):
    """Downsampled mask in the kernel's weight-column layout:
    Lm[p, g] = ds_flat[32p + g] (plain reshape)."""
    ds = mask_b[0, ::PATCH, ::PATCH].reshape(-1)           # (4096,)
    return np.ascontiguousarray(ds.reshape(P, T))


def make_in_maps(ep_mask_embed, ep_mask, query_mask_embed, query_mask):
    in_maps = []
    for b in range(B):
        in_maps.append({
            "ep_embed": np.ascontiguousarray(ep_mask_embed[b]),
            "q_embed": np.ascontiguousarray(query_mask_embed[b]),
            "lm": np.concatenate(
                [_perm_mask(ep_mask[b]), _perm_mask(query_mask[b])], axis=1),
        })
    return in_maps


def finalize(results):
    """results: list of 8 dicts with out_cnt [2,2], out_q [2,512],
    out_ew [2,256] -> full (3,) output (final epilogue in float64)."""
    pos = np.zeros(B)
    neg = np.zeros(B)
    for b, r in enumerate(results):
        cnt = np.asarray(r["out_cnt"], dtype=np.float64)   # [:,0]=ep, [:,1]=q
        qws = np.asarray(r["out_q"], dtype=np.float64)     # [qw | qsw]
        ew = np.asarray(r["out_ew"], dtype=np.float64)
        qw, qsw = qws[:, 0:C], qws[:, C:2 * C]
        ctr = ew / (cnt[:, 0:1] + 0.1)
        num = (qsw.sum(1) - 2.0 * (ctr * qw).sum(1)
               + cnt[:, 1] * (ctr * ctr).sum(1))
        loss = num / (np.maximum(cnt[:, 1], 1.0) * C) * np.minimum(cnt[:, 1], 1.0)
        pos[b], neg[b] = loss[0], loss[1]
    return np.array(
        [(pos + neg).mean(), pos.mean(), neg.mean()], dtype=np.float32
    )


def kernel(ep_mask_embed, ep_mask, query_mask_embed, query_mask):
    ep_mask_embed = np.asarray(ep_mask_embed, dtype=np.float32)
    ep_mask = np.asarray(ep_mask, dtype=np.float32)
    query_mask_embed = np.asarray(query_mask_embed, dtype=np.float32)
    query_mask = np.asarray(query_mask, dtype=np.float32)

    nc = get_nc()
    in_maps = make_in_maps(ep_mask_embed, ep_mask, query_mask_embed, query_mask)
    res = run_bass_kernel_spmd(nc, in_maps, list(range(B)))
    return finalize(res.results)
